# revision 4
# baseline (speedup 1.0000x reference)
"""Trainium2 Bass kernel for nn_CGNN (3-layer GNN message passing).

Math per layer:  prop = A @ h  (A sparse COO: out[row] += C * h[col]);
z = prop @ W + b; if not last: h' = l2norm_rows(relu(z)).

Distribution: destination-node sharding across 8 cores (6272 rows each,
49 tiles of 128). Each core gathers source rows h[col] for its edges with
the custom SWDGE dma_gather (4 queues), then performs the segment-sum as
PE matmuls:  propT[f, d] = sum_e G[e, f] * S[e, d]   (S built on host,
C folded in, bf16; edges grouped per dest tile, deduped by source, padded
to 128-chunks, uniform group sizes across cores so one SPMD program
serves all 8 cores).  h is AllGathered between layers (bf16).

Self-contained: hardcodes all shapes from the problem spec.
"""
import os
import sys
import types

import numpy as np
import ml_dtypes

# ---------------------------------------------------------------- constants
N = 50000
E = 800000
D = 128
NCLS = 64
NCORES = 8
P = 128
PAD_N = 50176            # 8 * 6272
SHARD = PAD_N // NCORES  # 6272
NT = SHARD // P          # 49 dest tiles per core
HALF = PAD_N // 2        # 25088 (int16 gather index limit is 32767)
BATCH_CH = 32            # chunks per gather call -> 4096 idx
BATCH = BATCH_CH * P     # 4096
NQ = 4                   # SWDGE queues
EPS = 1e-12

bf16 = ml_dtypes.bfloat16


# ---------------------------------------------------------------- host prep
def _prepare(edge_index, C_vals):
    """Build per-core gather index streams + packed S matrices + a
    core-uniform schedule.

    Returns dict with:
      nl, nh        : [NT] chunks per tile for lo/hi stream (uniform)
      nchunk        : total S chunks per core
      nbl, nbh      : gather batches per stream
      idx_lo, idx_hi: [NCORES][128, Llo/16] int16 wrapped indices
      s_mat         : [NCORES][128, nchunk, 128] bf16
    """
    row = np.asarray(edge_index[0], dtype=np.int64)
    col = np.asarray(edge_index[1], dtype=np.int64)
    C = np.asarray(C_vals, dtype=np.float32)

    core = row // SHARD
    tile_of = (row % SHARD) // P
    dloc = row % P
    half = (col >= HALF).astype(np.int64)

    # group key: (core, tile, half); within group dedup cols
    # first pass: unique source count per (c, t, h)
    ucount = np.zeros((NCORES, NT, 2), np.int64)
    groups = {}
    key = ((core * NT + tile_of) * 2 + half)
    order = np.argsort(key, kind="stable")
    ks = key[order]
    bounds = np.searchsorted(ks, np.arange(NCORES * NT * 2 + 1))
    for c in range(NCORES):
        for t in range(NT):
            for h in (0, 1):
                k = (c * NT + t) * 2 + h
                sel = order[bounds[k]:bounds[k + 1]]
                cols_g = col[sel]
                u, inv = np.unique(cols_g, return_inverse=True)
                groups[(c, t, h)] = (u, inv, dloc[sel], C[sel])
                ucount[c, t, h] = len(u)

    # uniform padded group sizes (multiples of P, max over cores)
    gsz = ((ucount.max(axis=0) + P - 1) // P) * P  # [NT, 2]
    nl = (gsz[:, 0] // P).astype(np.int64)
    nh = (gsz[:, 1] // P).astype(np.int64)
    nchunk = int((nl + nh).sum())
    s_off = np.concatenate([[0], np.cumsum(nl + nh)])[:NT]
    lo_off = np.concatenate([[0], np.cumsum(nl)])[:NT]
    hi_off = np.concatenate([[0], np.cumsum(nh)])[:NT]
    llo_ch = int(nl.sum())
    lhi_ch = int(nh.sum())
    nbl = -(-llo_ch // BATCH_CH)
    nbh = -(-lhi_ch // BATCH_CH)

    idx_lo_all, idx_hi_all, s_all = [], [], []
    for c in range(NCORES):
        s_mat = np.zeros((P, nchunk, P), np.float32)
        str_lo = np.zeros(nbl * BATCH, np.int64)
        str_hi = np.zeros(nbh * BATCH, np.int64)
        for t in range(NT):
            for h in (0, 1):
                u, inv, dl, cv = groups[(c, t, h)]
                base_s = (s_off[t] + (0 if h == 0 else nl[t])) * P
                r = base_s + inv
                np.add.at(s_mat, (r % P, r // P, dl), cv)
                stream = str_lo if h == 0 else str_hi
                boff = (lo_off[t] if h == 0 else hi_off[t]) * P
                stream[boff:boff + len(u)] = u - (0 if h == 0 else HALF)
        s_all.append(s_mat.astype(bf16))
        idx_lo_all.append(_wrap_idxs(str_lo))
        idx_hi_all.append(_wrap_idxs(str_hi))

    return {
        "nl": nl, "nh": nh, "nchunk": nchunk, "nbl": nbl, "nbh": nbh,
        "s_off": s_off, "lo_off": lo_off, "hi_off": hi_off,
        "idx_lo": idx_lo_all, "idx_hi": idx_hi_all, "s_mat": s_all,
    }


def _wrap_idxs(idx):
    """[L] -> [128, L/16] int16 wrapped (pos i = s*16 + p), replicated x8."""
    n = idx.shape[0]
    assert n % 16 == 0
    w = idx.astype(np.int16).reshape(n // 16, 16).T
    return np.ascontiguousarray(np.tile(w, (8, 1)))


# ---------------------------------------------------------------- device
def _build(sched):
    import concourse.bass as bass
    import concourse.bacc as bacc
    import concourse.mybir as mybir
    import concourse.tile as tile
    from concourse import library_config

    nl, nh = sched["nl"], sched["nh"]
    nchunk, nbl, nbh = sched["nchunk"], sched["nbl"], sched["nbh"]
    s_off, lo_off, hi_off = sched["s_off"], sched["lo_off"], sched["hi_off"]
    nsb = -(-nchunk // BATCH_CH)

    f32 = mybir.dt.float32
    b16 = mybir.dt.bfloat16

    nc = bacc.Bacc("TRN2", num_devices=NCORES, num_swdge_queues=NQ)
    xbf = nc.dram_tensor("xbf", [PAD_N, D], b16, kind="ExternalInput")
    s_in = nc.dram_tensor("s_mat", [P, nchunk, P], b16, kind="ExternalInput")
    ilo = nc.dram_tensor("idx_lo", [P, nbl * BATCH // 16], mybir.dt.int16,
                         kind="ExternalInput")
    ihi = nc.dram_tensor("idx_hi", [P, nbh * BATCH // 16], mybir.dt.int16,
                         kind="ExternalInput")
    w_in = [nc.dram_tensor(f"W{i+1}", [D, D if i < 2 else NCLS], f32,
                           kind="ExternalInput") for i in range(3)]
    b_in = [nc.dram_tensor(f"b{i+1}", [1, D if i < 2 else NCLS], f32,
                           kind="ExternalInput") for i in range(3)]
    out_t = nc.dram_tensor("out", [SHARD, NCLS], f32, kind="ExternalOutput")
    debug = bool(int(os.environ.get("GNN_DEBUG", "0")))
    if debug:
        dbg_h = [nc.dram_tensor(f"dbg_h{l+1}", [SHARD, D], b16,
                                kind="ExternalOutput") for l in range(2)]
        dbg_ag = [nc.dram_tensor(f"dbg_ag{l+1}", [PAD_N, D], b16,
                                 kind="ExternalOutput") for l in range(2)]
        dbg_prop = nc.dram_tensor("dbg_prop", [P, P], f32, kind="ExternalOutput")
        dbg_g = nc.dram_tensor("dbg_g", [P, BATCH_CH, D], b16, kind="ExternalOutput")

    with tile.TileContext(nc) as tc:
        nc.gpsimd.load_library(library_config.mlp)
        with (
            tc.tile_pool(name="dram", bufs=1, space="DRAM") as dram,
            tc.tile_pool(name="singles", bufs=1) as singles,
            tc.tile_pool(name="glo", bufs=4) as glo_pool,
            tc.tile_pool(name="ghi", bufs=4) as ghi_pool,
            tc.tile_pool(name="sbat", bufs=4) as sb_pool,
            tc.tile_pool(name="work", bufs=3) as work,
            tc.tile_pool(name="psum_p", bufs=3, space="PSUM") as psum_p,
            tc.tile_pool(name="psum_z", bufs=3, space="PSUM") as psum_z,
        ):
            ag_in = [dram.tile([SHARD, D], b16, name=f"ag_in{l}", tag=f"ag_in{l}") for l in range(2)]
            ag_out = [dram.tile([PAD_N, D], b16, name=f"ag_out{l}", tag=f"ag_out{l}") for l in range(2)]

            idx_lo_t = singles.tile([P, nbl * BATCH // 16], mybir.dt.int16,
                                    tag="idxlo")
            idx_hi_t = singles.tile([P, nbh * BATCH // 16], mybir.dt.int16,
                                    tag="idxhi")
            nc.sync.dma_start(out=idx_lo_t[:], in_=ilo[:])
            nc.sync.dma_start(out=idx_hi_t[:], in_=ihi[:])
            w_t, b_t = [], []
            for i in range(3):
                nout = D if i < 2 else NCLS
                wt = singles.tile([D, nout], f32, name=f"w{i}", tag=f"w{i}")
                bt = singles.tile([1, nout], f32, name=f"b{i}", tag=f"b{i}")
                nc.sync.dma_start(out=wt[:], in_=w_in[i][:])
                nc.sync.dma_start(out=bt[:], in_=b_in[i][:])
                w_t.append(wt)
                b_t.append(bt)
            ones_t = singles.tile([1, P], f32, tag="ones")
            nc.vector.memset(ones_t[:], 1.0)

            qrr = [0]

            def gather_stream(table_ap, idx_tile, nbatch, pool, off):
                bufs = []
                for b in range(nbatch):
                    g = pool.tile([P, BATCH_CH, D], b16)
                    nc.gpsimd.dma_gather(
                        g[:], table_ap,
                        idx_tile[:, (b * BATCH // 16):((b + 1) * BATCH // 16)],
                        BATCH, BATCH, D,
                        single_packet=False, queue_num=qrr[0] % NQ,
                    )
                    qrr[0] += 1
                    bufs.append(g)
                return bufs

            for l in range(3):
                nout = D if l < 2 else NCLS
                if l == 0:
                    tbl = xbf
                else:
                    tbl = ag_out[l - 1]
                tbl_lo = tbl[0:HALF, :]
                tbl_hi = tbl[HALF:PAD_N, :]

                g_lo = gather_stream(tbl_lo, idx_lo_t, nbl, glo_pool, 0)
                if debug and l == 1:
                    nc.sync.dma_start(out=dbg_g[:], in_=g_lo[0][:])
                g_hi = gather_stream(tbl_hi, idx_hi_t, nbh, ghi_pool, 0)
                s_bufs = []
                for b in range(nsb):
                    c0 = b * BATCH_CH
                    c1 = min(nchunk, c0 + BATCH_CH)
                    sb = sb_pool.tile([P, BATCH_CH, P], b16)
                    nc.sync.dma_start(out=sb[:, 0:(c1 - c0), :],
                                      in_=s_in[:, c0:c1, :])
                    s_bufs.append(sb)

                for t in range(NT):
                    pp = psum_p.tile([P, P], f32)
                    ntot = int(nl[t] + nh[t])
                    ci = 0
                    for h in (0, 1):
                        nch = int(nl[t]) if h == 0 else int(nh[t])
                        stream_base = int(lo_off[t]) if h == 0 else int(hi_off[t])
                        gb = g_lo if h == 0 else g_hi
                        for i in range(nch):
                            j = stream_base + i
                            k = int(s_off[t]) + ci
                            nc.tensor.matmul(
                                out=pp[:],
                                lhsT=gb[j // BATCH_CH][:, j % BATCH_CH, :],
                                rhs=s_bufs[k // BATCH_CH][:, k % BATCH_CH, :],
                                start=(ci == 0), stop=(ci == ntot - 1),
                            )
                            ci += 1
                    propT = work.tile([P, P], f32, tag="propT")
                    nc.vector.tensor_copy(out=propT[:], in_=pp[:])
                    if debug and l == 0 and t == 0:
                        nc.sync.dma_start(out=dbg_prop[:], in_=propT[:])
                    pz = psum_z.tile([P, nout], f32, tag="pz")
                    nc.tensor.matmul(out=pz[:], lhsT=propT[:], rhs=w_t[l][:],
                                     start=True, stop=False)
                    nc.tensor.matmul(out=pz[:], lhsT=ones_t[:], rhs=b_t[l][:],
                                     start=False, stop=True)
                    if l < 2:
                        ht = work.tile([P, D], f32, tag="ht")
                        nc.scalar.activation(
                            out=ht[:], in_=pz[:],
                            func=mybir.ActivationFunctionType.Relu)
                        sq = work.tile([P, D], f32, tag="sq")
                        ss = work.tile([P, 1], f32, tag="ss")
                        nc.vector.tensor_mul(out=sq[:], in0=ht[:], in1=ht[:])
                        nc.vector.tensor_reduce(
                            out=ss[:], in_=sq[:], axis=mybir.AxisListType.X,
                            op=mybir.AluOpType.add)
                        nc.scalar.activation(
                            out=ss[:], in_=ss[:],
                            func=mybir.ActivationFunctionType.Sqrt)
                        nc.vector.tensor_scalar_max(out=ss[:], in0=ss[:],
                                                    scalar1=float(EPS))
                        nc.vector.reciprocal(out=ss[:], in_=ss[:])
                        hb = work.tile([P, D], b16, tag="hb")
                        nc.vector.tensor_scalar_mul(out=hb[:], in0=ht[:],
                                                    scalar1=ss[:])
                        nc.sync.dma_start(
                            out=ag_in[l][t * P:(t + 1) * P, :], in_=hb[:])
                    else:
                        zt = work.tile([P, NCLS], f32, tag="zt")
                        nc.vector.tensor_copy(out=zt[:], in_=pz[:])
                        nc.sync.dma_start(
                            out=out_t[t * P:(t + 1) * P, :], in_=zt[:])
                if l < 2:
                    nc.gpsimd.collective_compute(
                        "AllGather",
                        mybir.AluOpType.bypass,
                        ins=[ag_in[l].opt()],
                        outs=[ag_out[l].opt()],
                        replica_groups=[list(range(NCORES))],
                    )
                    if debug:
                        nc.sync.dma_start(out=dbg_h[l][:], in_=ag_in[l][:])
                        nc.sync.dma_start(out=dbg_ag[l][:], in_=ag_out[l][:])
    nc.compile()
    return nc


_CACHE = {}


def _get_program(sched):
    key = (sched["nchunk"], sched["nbl"], sched["nbh"],
           tuple(sched["nl"]), tuple(sched["nh"]))
    if key not in _CACHE:
        _CACHE[key] = _build(sched)
    return _CACHE[key]


# ---------------------------------------------------------------- entry
def kernel(x, edge_index, C_vals, W1, b1, W2, b2, W3, b3):
    from concourse.bass_utils import run_bass_kernel_spmd

    x = np.asarray(x)
    sched = _prepare(edge_index, C_vals)
    nc = _get_program(sched)

    xbf = np.zeros((PAD_N, D), bf16)
    xbf[:N] = x.astype(bf16)
    common = {
        "xbf": xbf,
        "W1": np.asarray(W1, np.float32), "b1": np.asarray(b1, np.float32).reshape(1, D),
        "W2": np.asarray(W2, np.float32), "b2": np.asarray(b2, np.float32).reshape(1, D),
        "W3": np.asarray(W3, np.float32), "b3": np.asarray(b3, np.float32).reshape(1, NCLS),
    }
    in_maps = []
    for c in range(NCORES):
        m = dict(common)
        m["s_mat"] = sched["s_mat"][c]
        m["idx_lo"] = sched["idx_lo"][c]
        m["idx_hi"] = sched["idx_hi"][c]
        in_maps.append(m)

    trace = bool(int(os.environ.get("GNN_TRACE", "0")))
    kwargs = {}
    if trace:
        import trace_utils
        trace_utils.install()
        kwargs = dict(trace=True, tmpdir="/tmp/gnn_trace")

    res = run_bass_kernel_spmd(nc, in_maps, core_ids=list(range(NCORES)),
                               **kwargs)
    if trace and res.exec_time_ns is not None:
        print(f"HW exec time: {res.exec_time_ns} ns")

    out = np.concatenate([res.results[c]["out"] for c in range(NCORES)], axis=0)
    return np.ascontiguousarray(out[:N])


# revision 6
# speedup vs baseline: 1.0135x; 1.0135x over previous
"""Trainium2 Bass kernel for nn_CGNN (3-layer GNN message passing).

Math per layer:  prop = A @ h  (A sparse COO: out[row] += C * h[col]);
z = prop @ W + b; if not last: h' = l2norm_rows(relu(z)).

Distribution: destination-node sharding across 8 cores (6272 rows each,
49 tiles of 128). Each core gathers source rows h[col] for its edges with
the custom SWDGE dma_gather (4 queues), then performs the segment-sum as
PE matmuls:  propT[f, d] = sum_e G[e, f] * S[e, d]   (S built on host,
C folded in, bf16; edges grouped per dest tile, deduped by source, padded
to 128-chunks, uniform group sizes across cores so one SPMD program
serves all 8 cores).  h is AllGathered between layers (bf16).

Self-contained: hardcodes all shapes from the problem spec.
"""
import os
import sys
import types

import numpy as np
import ml_dtypes

# ---------------------------------------------------------------- constants
N = 50000
E = 800000
D = 128
NCLS = 64
NCORES = 8
P = 128
PAD_N = 50176            # 8 * 6272
SHARD = PAD_N // NCORES  # 6272
NT = SHARD // P          # 49 dest tiles per core
HALF = PAD_N // 2        # 25088 (int16 gather index limit is 32767)
BATCH_CH = 32            # chunks per gather call -> 4096 idx
BATCH = BATCH_CH * P     # 4096
NQ = 4                   # SWDGE queues
EPS = 1e-12

bf16 = ml_dtypes.bfloat16


# ---------------------------------------------------------------- host prep
def _prepare(edge_index, C_vals):
    """Build per-core gather index streams + packed S matrices + a
    core-uniform schedule.

    Returns dict with:
      nl, nh        : [NT] chunks per tile for lo/hi stream (uniform)
      nchunk        : total S chunks per core
      nbl, nbh      : gather batches per stream
      idx_lo, idx_hi: [NCORES][128, Llo/16] int16 wrapped indices
      s_mat         : [NCORES][128, nchunk, 128] bf16
    """
    row = np.asarray(edge_index[0], dtype=np.int64)
    col = np.asarray(edge_index[1], dtype=np.int64)
    C = np.asarray(C_vals, dtype=np.float32)

    core = row // SHARD
    tile_of = (row % SHARD) // P
    dloc = row % P
    half = (col >= HALF).astype(np.int64)

    # group key: (core, tile, half); within group dedup cols
    # first pass: unique source count per (c, t, h)
    ucount = np.zeros((NCORES, NT, 2), np.int64)
    groups = {}
    key = ((core * NT + tile_of) * 2 + half)
    order = np.argsort(key, kind="stable")
    ks = key[order]
    bounds = np.searchsorted(ks, np.arange(NCORES * NT * 2 + 1))
    for c in range(NCORES):
        for t in range(NT):
            for h in (0, 1):
                k = (c * NT + t) * 2 + h
                sel = order[bounds[k]:bounds[k + 1]]
                cols_g = col[sel]
                u, inv = np.unique(cols_g, return_inverse=True)
                groups[(c, t, h)] = (u, inv, dloc[sel], C[sel])
                ucount[c, t, h] = len(u)

    # uniform padded group sizes (multiples of P, max over cores)
    gsz = ((ucount.max(axis=0) + P - 1) // P) * P  # [NT, 2]
    nl = (gsz[:, 0] // P).astype(np.int64)
    nh = (gsz[:, 1] // P).astype(np.int64)
    nchunk = int((nl + nh).sum())
    s_off = np.concatenate([[0], np.cumsum(nl + nh)])[:NT]
    lo_off = np.concatenate([[0], np.cumsum(nl)])[:NT]
    hi_off = np.concatenate([[0], np.cumsum(nh)])[:NT]
    llo_ch = int(nl.sum())
    lhi_ch = int(nh.sum())
    nbl = -(-llo_ch // BATCH_CH)
    nbh = -(-lhi_ch // BATCH_CH)

    idx_lo_all, idx_hi_all, s_all = [], [], []
    for c in range(NCORES):
        s_mat = np.zeros((P, nchunk, P), np.float32)
        str_lo = np.zeros(nbl * BATCH, np.int64)
        str_hi = np.zeros(nbh * BATCH, np.int64)
        for t in range(NT):
            for h in (0, 1):
                u, inv, dl, cv = groups[(c, t, h)]
                base_s = (s_off[t] + (0 if h == 0 else nl[t])) * P
                r = base_s + inv
                np.add.at(s_mat, (r % P, r // P, dl), cv)
                stream = str_lo if h == 0 else str_hi
                boff = (lo_off[t] if h == 0 else hi_off[t]) * P
                stream[boff:boff + len(u)] = u - (0 if h == 0 else HALF)
        s_all.append(s_mat.astype(bf16))
        idx_lo_all.append(_wrap_idxs(str_lo))
        idx_hi_all.append(_wrap_idxs(str_hi))

    return {
        "nl": nl, "nh": nh, "nchunk": nchunk, "nbl": nbl, "nbh": nbh,
        "s_off": s_off, "lo_off": lo_off, "hi_off": hi_off,
        "idx_lo": idx_lo_all, "idx_hi": idx_hi_all, "s_mat": s_all,
    }


def _wrap_idxs(idx):
    """[L] -> [128, L/16] int16 wrapped (pos i = s*16 + p), replicated x8."""
    n = idx.shape[0]
    assert n % 16 == 0
    w = idx.astype(np.int16).reshape(n // 16, 16).T
    return np.ascontiguousarray(np.tile(w, (8, 1)))


# ---------------------------------------------------------------- device
def _build(sched):
    import concourse.bass as bass
    import concourse.bacc as bacc
    import concourse.mybir as mybir
    import concourse.tile as tile
    from concourse import library_config

    nl, nh = sched["nl"], sched["nh"]
    nchunk, nbl, nbh = sched["nchunk"], sched["nbl"], sched["nbh"]
    s_off, lo_off, hi_off = sched["s_off"], sched["lo_off"], sched["hi_off"]
    nsb = -(-nchunk // BATCH_CH)

    f32 = mybir.dt.float32
    b16 = mybir.dt.bfloat16

    nc = bacc.Bacc("TRN2", num_devices=NCORES, num_swdge_queues=NQ)
    xbf = nc.dram_tensor("xbf", [PAD_N, D], b16, kind="ExternalInput")
    s_in = nc.dram_tensor("s_mat", [P, nchunk, P], b16, kind="ExternalInput")
    ilo = nc.dram_tensor("idx_lo", [P, nbl * BATCH // 16], mybir.dt.int16,
                         kind="ExternalInput")
    ihi = nc.dram_tensor("idx_hi", [P, nbh * BATCH // 16], mybir.dt.int16,
                         kind="ExternalInput")
    w_in = [nc.dram_tensor(f"W{i+1}", [D, D if i < 2 else NCLS], b16,
                           kind="ExternalInput") for i in range(3)]
    b_in = [nc.dram_tensor(f"b{i+1}", [1, D if i < 2 else NCLS], b16,
                           kind="ExternalInput") for i in range(3)]
    out_t = nc.dram_tensor("out", [SHARD, NCLS], f32, kind="ExternalOutput")
    debug = bool(int(os.environ.get("GNN_DEBUG", "0")))
    if debug:
        dbg_h = [nc.dram_tensor(f"dbg_h{l+1}", [SHARD, D], b16,
                                kind="ExternalOutput") for l in range(2)]
        dbg_ag = [nc.dram_tensor(f"dbg_ag{l+1}", [PAD_N, D], b16,
                                 kind="ExternalOutput") for l in range(2)]
        dbg_prop = nc.dram_tensor("dbg_prop", [P, P], b16, kind="ExternalOutput")
        dbg_g = nc.dram_tensor("dbg_g", [P, BATCH_CH, D], b16, kind="ExternalOutput")

    with tile.TileContext(nc) as tc:
        nc.gpsimd.load_library(library_config.mlp)
        with (
            tc.tile_pool(name="dram", bufs=1, space="DRAM") as dram,
            tc.tile_pool(name="singles", bufs=1) as singles,
            tc.tile_pool(name="glo", bufs=6) as glo_pool,
            tc.tile_pool(name="ghi", bufs=6) as ghi_pool,
            tc.tile_pool(name="sbat", bufs=4) as sb_pool,
            tc.tile_pool(name="work", bufs=6) as work,
            tc.tile_pool(name="psum_p", bufs=3, space="PSUM") as psum_p,
            tc.tile_pool(name="psum_z", bufs=3, space="PSUM") as psum_z,
        ):
            ag_in = [dram.tile([SHARD, D], b16, name=f"ag_in{l}", tag=f"ag_in{l}") for l in range(2)]
            ag_out = [dram.tile([PAD_N, D], b16, name=f"ag_out{l}", tag=f"ag_out{l}") for l in range(2)]

            idx_lo_t = singles.tile([P, nbl * BATCH // 16], mybir.dt.int16,
                                    tag="idxlo")
            idx_hi_t = singles.tile([P, nbh * BATCH // 16], mybir.dt.int16,
                                    tag="idxhi")
            nc.sync.dma_start(out=idx_lo_t[:], in_=ilo[:])
            nc.sync.dma_start(out=idx_hi_t[:], in_=ihi[:])
            w_t, b_t = [], []
            for i in range(3):
                nout = D if i < 2 else NCLS
                wt = singles.tile([D, nout], b16, name=f"w{i}", tag=f"w{i}")
                bt = singles.tile([1, nout], b16, name=f"b{i}", tag=f"b{i}")
                nc.sync.dma_start(out=wt[:], in_=w_in[i][:])
                nc.sync.dma_start(out=bt[:], in_=b_in[i][:])
                w_t.append(wt)
                b_t.append(bt)
            ones_t = singles.tile([1, P], b16, tag="ones")
            nc.vector.memset(ones_t[:], 1.0)

            qrr = [0]

            def gather_stream(table_ap, idx_tile, nbatch, pool, off):
                bufs = []
                for b in range(nbatch):
                    g = pool.tile([P, BATCH_CH, D], b16)
                    nc.gpsimd.dma_gather(
                        g[:], table_ap,
                        idx_tile[:, (b * BATCH // 16):((b + 1) * BATCH // 16)],
                        BATCH, BATCH, D,
                        single_packet=False, queue_num=qrr[0] % NQ,
                    )
                    qrr[0] += 1
                    bufs.append(g)
                return bufs

            for l in range(3):
                nout = D if l < 2 else NCLS
                if l == 0:
                    tbl = xbf
                else:
                    tbl = ag_out[l - 1]
                tbl_lo = tbl[0:HALF, :]
                tbl_hi = tbl[HALF:PAD_N, :]

                g_lo = gather_stream(tbl_lo, idx_lo_t, nbl, glo_pool, 0)
                if debug and l == 1:
                    nc.sync.dma_start(out=dbg_g[:], in_=g_lo[0][:])
                g_hi = gather_stream(tbl_hi, idx_hi_t, nbh, ghi_pool, 0)
                s_bufs = []
                for b in range(nsb):
                    c0 = b * BATCH_CH
                    c1 = min(nchunk, c0 + BATCH_CH)
                    sb = sb_pool.tile([P, BATCH_CH, P], b16)
                    nc.sync.dma_start(out=sb[:, 0:(c1 - c0), :],
                                      in_=s_in[:, c0:c1, :])
                    s_bufs.append(sb)

                for t in range(NT):
                    pp = psum_p.tile([P, P], f32)
                    ntot = int(nl[t] + nh[t])
                    ci = 0
                    for h in (0, 1):
                        nch = int(nl[t]) if h == 0 else int(nh[t])
                        stream_base = int(lo_off[t]) if h == 0 else int(hi_off[t])
                        gb = g_lo if h == 0 else g_hi
                        for i in range(nch):
                            j = stream_base + i
                            k = int(s_off[t]) + ci
                            nc.tensor.matmul(
                                out=pp[:],
                                lhsT=gb[j // BATCH_CH][:, j % BATCH_CH, :],
                                rhs=s_bufs[k // BATCH_CH][:, k % BATCH_CH, :],
                                start=(ci == 0), stop=(ci == ntot - 1),
                            )
                            ci += 1
                    propT = work.tile([P, P], b16, tag="propT")
                    nc.vector.tensor_copy(out=propT[:], in_=pp[:])
                    if debug and l == 0 and t == 0:
                        nc.sync.dma_start(out=dbg_prop[:], in_=propT[:])
                    pz = psum_z.tile([P, nout], f32, tag="pz")
                    nc.tensor.matmul(out=pz[:], lhsT=propT[:], rhs=w_t[l][:],
                                     start=True, stop=False)
                    nc.tensor.matmul(out=pz[:], lhsT=ones_t[:], rhs=b_t[l][:],
                                     start=False, stop=True)
                    if l < 2:
                        ht = work.tile([P, D], f32, tag="ht")
                        nc.scalar.activation(
                            out=ht[:], in_=pz[:],
                            func=mybir.ActivationFunctionType.Relu)
                        sq = work.tile([P, D], f32, tag="sq")
                        ss = work.tile([P, 1], f32, tag="ss")
                        nc.scalar.activation(
                            out=sq[:], in_=ht[:],
                            func=mybir.ActivationFunctionType.Square,
                            accum_out=ss[:])
                        nc.scalar.activation(
                            out=ss[:], in_=ss[:],
                            func=mybir.ActivationFunctionType.Sqrt)
                        nc.vector.tensor_scalar_max(out=ss[:], in0=ss[:],
                                                    scalar1=float(EPS))
                        nc.vector.reciprocal(out=ss[:], in_=ss[:])
                        hb = work.tile([P, D], b16, tag="hb")
                        nc.scalar.activation(
                            out=hb[:], in_=ht[:],
                            func=mybir.ActivationFunctionType.Copy,
                            scale=ss[:])
                        nc.sync.dma_start(
                            out=ag_in[l][t * P:(t + 1) * P, :], in_=hb[:])
                    else:
                        zt = work.tile([P, NCLS], f32, tag="zt")
                        nc.vector.tensor_copy(out=zt[:], in_=pz[:])
                        nc.sync.dma_start(
                            out=out_t[t * P:(t + 1) * P, :], in_=zt[:])
                if l < 2:
                    nc.gpsimd.collective_compute(
                        "AllGather",
                        mybir.AluOpType.bypass,
                        ins=[ag_in[l].opt()],
                        outs=[ag_out[l].opt()],
                        replica_groups=[list(range(NCORES))],
                    )
                    if debug:
                        nc.sync.dma_start(out=dbg_h[l][:], in_=ag_in[l][:])
                        nc.sync.dma_start(out=dbg_ag[l][:], in_=ag_out[l][:])
    nc.compile()
    return nc


_CACHE = {}


def _get_program(sched):
    key = (sched["nchunk"], sched["nbl"], sched["nbh"],
           tuple(sched["nl"]), tuple(sched["nh"]))
    if key not in _CACHE:
        _CACHE[key] = _build(sched)
    return _CACHE[key]


# ---------------------------------------------------------------- entry
def kernel(x, edge_index, C_vals, W1, b1, W2, b2, W3, b3):
    from concourse.bass_utils import run_bass_kernel_spmd

    x = np.asarray(x)
    sched = _prepare(edge_index, C_vals)
    nc = _get_program(sched)

    xbf = np.zeros((PAD_N, D), bf16)
    xbf[:N] = x.astype(bf16)
    common = {
        "xbf": xbf,
        "W1": np.asarray(W1).astype(bf16), "b1": np.asarray(b1).astype(bf16).reshape(1, D),
        "W2": np.asarray(W2).astype(bf16), "b2": np.asarray(b2).astype(bf16).reshape(1, D),
        "W3": np.asarray(W3).astype(bf16), "b3": np.asarray(b3).astype(bf16).reshape(1, NCLS),
    }
    in_maps = []
    for c in range(NCORES):
        m = dict(common)
        m["s_mat"] = sched["s_mat"][c]
        m["idx_lo"] = sched["idx_lo"][c]
        m["idx_hi"] = sched["idx_hi"][c]
        in_maps.append(m)

    trace = bool(int(os.environ.get("GNN_TRACE", "0")))
    kwargs = {}
    if trace:
        import trace_utils
        trace_utils.install()
        kwargs = dict(trace=True, tmpdir="/tmp/gnn_trace")

    res = run_bass_kernel_spmd(nc, in_maps, core_ids=list(range(NCORES)),
                               **kwargs)
    if trace and res.exec_time_ns is not None:
        print(f"HW exec time: {res.exec_time_ns} ns")

    out = np.concatenate([res.results[c]["out"] for c in range(NCORES)], axis=0)
    return np.ascontiguousarray(out[:N])


# revision 7
# speedup vs baseline: 1.2329x; 1.2165x over previous
"""Trainium2 Bass kernel for nn_CGNN (3-layer GNN message passing).

Math per layer:  prop = A @ h  (A sparse COO: out[row] += C * h[col]);
z = prop @ W + b; if not last: h' = l2norm_rows(relu(z)).

Distribution: destination-node sharding across 8 cores (6272 rows each,
49 tiles of 128). Each core gathers source rows h[col] for its edges with
the custom SWDGE dma_gather (4 queues), then performs the segment-sum as
PE matmuls:  propT[f, d] = sum_e G[e, f] * S[e, d]   (S built on host,
C folded in, bf16; edges grouped per dest tile, deduped by source, padded
to 128-chunks, uniform group sizes across cores so one SPMD program
serves all 8 cores).  h is AllGathered between layers (bf16).

Self-contained: hardcodes all shapes from the problem spec.
"""
import os
import sys
import types

import numpy as np
import ml_dtypes

# ---------------------------------------------------------------- constants
N = 50000
E = 800000
D = 128
NCLS = 64
NCORES = 8
P = 128
PAD_N = 50176            # 8 * 6272
SHARD = PAD_N // NCORES  # 6272
NT = SHARD // P          # 49 dest tiles per core
HALF = PAD_N // 2        # 25088 (int16 gather index limit is 32767)
BATCH_CH = 32            # chunks per gather call -> 4096 idx
BATCH = BATCH_CH * P     # 4096
NQ = 4                   # SWDGE queues
EPS = 1e-12

bf16 = ml_dtypes.bfloat16


# ---------------------------------------------------------------- host prep
def _prepare(edge_index, C_vals):
    """Build per-core gather index streams + packed S matrices + a
    core-uniform schedule.

    Returns dict with:
      nl, nh        : [NT] chunks per tile for lo/hi stream (uniform)
      nchunk        : total S chunks per core
      nbl, nbh      : gather batches per stream
      idx_lo, idx_hi: [NCORES][128, Llo/16] int16 wrapped indices
      s_mat         : [NCORES][128, nchunk, 128] bf16
    """
    row = np.asarray(edge_index[0], dtype=np.int64)
    col = np.asarray(edge_index[1], dtype=np.int64)
    C = np.asarray(C_vals, dtype=np.float32)

    core = row // SHARD
    tile_of = (row % SHARD) // P
    dloc = row % P
    half = (col >= HALF).astype(np.int64)

    # group key: (core, tile, half); within group dedup cols
    # first pass: unique source count per (c, t, h)
    ucount = np.zeros((NCORES, NT, 2), np.int64)
    groups = {}
    key = ((core * NT + tile_of) * 2 + half)
    order = np.argsort(key, kind="stable")
    ks = key[order]
    bounds = np.searchsorted(ks, np.arange(NCORES * NT * 2 + 1))
    for c in range(NCORES):
        for t in range(NT):
            for h in (0, 1):
                k = (c * NT + t) * 2 + h
                sel = order[bounds[k]:bounds[k + 1]]
                cols_g = col[sel]
                u, inv = np.unique(cols_g, return_inverse=True)
                groups[(c, t, h)] = (u, inv, dloc[sel], C[sel])
                ucount[c, t, h] = len(u)

    # uniform padded group sizes (multiples of P, max over cores)
    gsz = ((ucount.max(axis=0) + P - 1) // P) * P  # [NT, 2]
    nl = (gsz[:, 0] // P).astype(np.int64)
    nh = (gsz[:, 1] // P).astype(np.int64)
    nchunk = int((nl + nh).sum())
    s_off = np.concatenate([[0], np.cumsum(nl + nh)])[:NT]
    lo_off = np.concatenate([[0], np.cumsum(nl)])[:NT]
    hi_off = np.concatenate([[0], np.cumsum(nh)])[:NT]
    llo_ch = int(nl.sum())
    lhi_ch = int(nh.sum())
    nbl = -(-llo_ch // BATCH_CH)
    nbh = -(-lhi_ch // BATCH_CH)

    idx_lo_all, idx_hi_all, s_all = [], [], []
    for c in range(NCORES):
        s_mat = np.zeros((P, nchunk, P), np.float32)
        str_lo = np.zeros(nbl * BATCH, np.int64)
        str_hi = np.zeros(nbh * BATCH, np.int64)
        for t in range(NT):
            for h in (0, 1):
                u, inv, dl, cv = groups[(c, t, h)]
                base_s = (s_off[t] + (0 if h == 0 else nl[t])) * P
                r = base_s + inv
                np.add.at(s_mat, (r % P, r // P, dl), cv)
                stream = str_lo if h == 0 else str_hi
                boff = (lo_off[t] if h == 0 else hi_off[t]) * P
                stream[boff:boff + len(u)] = u - (0 if h == 0 else HALF)
        s_all.append(s_mat.astype(bf16))
        idx_lo_all.append(_wrap_idxs(str_lo))
        idx_hi_all.append(_wrap_idxs(str_hi))

    return {
        "nl": nl, "nh": nh, "nchunk": nchunk, "nbl": nbl, "nbh": nbh,
        "s_off": s_off, "lo_off": lo_off, "hi_off": hi_off,
        "idx_lo": idx_lo_all, "idx_hi": idx_hi_all, "s_mat": s_all,
    }


def _wrap_idxs(idx):
    """[L] -> [128, L/16] int16 wrapped (pos i = s*16 + p), replicated x8."""
    n = idx.shape[0]
    assert n % 16 == 0
    w = idx.astype(np.int16).reshape(n // 16, 16).T
    return np.ascontiguousarray(np.tile(w, (8, 1)))


# ---------------------------------------------------------------- device
def _build(sched):
    import concourse.bass as bass
    import concourse.bacc as bacc
    import concourse.mybir as mybir
    import concourse.tile as tile
    from concourse import library_config

    nl, nh = sched["nl"], sched["nh"]
    nchunk, nbl, nbh = sched["nchunk"], sched["nbl"], sched["nbh"]
    s_off, lo_off, hi_off = sched["s_off"], sched["lo_off"], sched["hi_off"]
    nsb = -(-nchunk // BATCH_CH)

    f32 = mybir.dt.float32
    b16 = mybir.dt.bfloat16

    nc = bacc.Bacc("TRN2", num_devices=NCORES, num_swdge_queues=NQ)
    xbf = nc.dram_tensor("xbf", [PAD_N, D], b16, kind="ExternalInput")
    s_in = nc.dram_tensor("s_mat", [P, nchunk, P], b16, kind="ExternalInput")
    ilo = nc.dram_tensor("idx_lo", [P, nbl * BATCH // 16], mybir.dt.int16,
                         kind="ExternalInput")
    ihi = nc.dram_tensor("idx_hi", [P, nbh * BATCH // 16], mybir.dt.int16,
                         kind="ExternalInput")
    w_in = [nc.dram_tensor(f"W{i+1}", [D, D if i < 2 else NCLS], b16,
                           kind="ExternalInput") for i in range(3)]
    b_in = [nc.dram_tensor(f"b{i+1}", [1, D if i < 2 else NCLS], b16,
                           kind="ExternalInput") for i in range(3)]
    out_t = nc.dram_tensor("out", [SHARD, NCLS], f32, kind="ExternalOutput")
    debug = bool(int(os.environ.get("GNN_DEBUG", "0")))
    if debug:
        dbg_h = [nc.dram_tensor(f"dbg_h{l+1}", [SHARD, D], b16,
                                kind="ExternalOutput") for l in range(2)]
        dbg_ag = [nc.dram_tensor(f"dbg_ag{l+1}", [PAD_N, D], b16,
                                 kind="ExternalOutput") for l in range(2)]
        dbg_prop = nc.dram_tensor("dbg_prop", [P, P], b16, kind="ExternalOutput")
        dbg_g = nc.dram_tensor("dbg_g", [P, BATCH_CH, D], b16, kind="ExternalOutput")

    with tile.TileContext(nc) as tc:
        nc.gpsimd.load_library(library_config.mlp)
        with (
            tc.tile_pool(name="dram", bufs=1, space="DRAM") as dram,
            tc.tile_pool(name="singles", bufs=1) as singles,
            tc.tile_pool(name="glo", bufs=6) as glo_pool,
            tc.tile_pool(name="ghi", bufs=6) as ghi_pool,
            tc.tile_pool(name="sbat", bufs=4) as sb_pool,
            tc.tile_pool(name="work", bufs=6) as work,
            tc.tile_pool(name="psum_p", bufs=3, space="PSUM") as psum_p,
            tc.tile_pool(name="psum_z", bufs=3, space="PSUM") as psum_z,
        ):
            ag_in = [dram.tile([SHARD, D], b16, name=f"ag_in{l}", tag=f"ag_in{l}") for l in range(2)]
            ag_out = [dram.tile([PAD_N, D], b16, name=f"ag_out{l}", tag=f"ag_out{l}") for l in range(2)]

            idx_lo_t = singles.tile([P, nbl * BATCH // 16], mybir.dt.int16,
                                    tag="idxlo")
            idx_hi_t = singles.tile([P, nbh * BATCH // 16], mybir.dt.int16,
                                    tag="idxhi")
            nc.sync.dma_start(out=idx_lo_t[:], in_=ilo[:])
            nc.sync.dma_start(out=idx_hi_t[:], in_=ihi[:])
            w_t, b_t = [], []
            for i in range(3):
                nout = D if i < 2 else NCLS
                wt = singles.tile([D, nout], b16, name=f"w{i}", tag=f"w{i}")
                bt = singles.tile([1, nout], b16, name=f"b{i}", tag=f"b{i}")
                nc.sync.dma_start(out=wt[:], in_=w_in[i][:])
                nc.sync.dma_start(out=bt[:], in_=b_in[i][:])
                w_t.append(wt)
                b_t.append(bt)
            ones_t = singles.tile([1, P], b16, tag="ones")
            nc.vector.memset(ones_t[:], 1.0)

            qrr = [0]

            def issue_gather(table_ap, idx_tile, b, pool):
                g = pool.tile([P, BATCH_CH, D], b16)
                nc.gpsimd.dma_gather(
                    g[:], table_ap,
                    idx_tile[:, (b * BATCH // 16):((b + 1) * BATCH // 16)],
                    BATCH, BATCH, D,
                    single_packet=False, queue_num=qrr[0] % NQ,
                )
                qrr[0] += 1
                return g

            for l in range(3):
                nout = D if l < 2 else NCLS
                if l == 0:
                    tbl = xbf
                else:
                    tbl = ag_out[l - 1]
                tbl_lo = tbl[0:HALF, :]
                tbl_hi = tbl[HALF:PAD_N, :]

                # interleave lo/hi gather issue so tiles complete progressively
                g_lo, g_hi = [], []
                for b in range(max(nbl, nbh)):
                    if b < nbl:
                        g_lo.append(issue_gather(tbl_lo, idx_lo_t, b, glo_pool))
                    if b < nbh:
                        g_hi.append(issue_gather(tbl_hi, idx_hi_t, b, ghi_pool))
                if debug and l == 1:
                    nc.sync.dma_start(out=dbg_g[:], in_=g_lo[0][:])
                s_bufs = []
                for b in range(nsb):
                    c0 = b * BATCH_CH
                    c1 = min(nchunk, c0 + BATCH_CH)
                    sb = sb_pool.tile([P, BATCH_CH, P], b16)
                    nc.sync.dma_start(out=sb[:, 0:(c1 - c0), :],
                                      in_=s_in[:, c0:c1, :])
                    s_bufs.append(sb)

                for t in range(NT):
                    pp = psum_p.tile([P, P], f32)
                    ntot = int(nl[t] + nh[t])
                    ci = 0
                    for h in (0, 1):
                        nch = int(nl[t]) if h == 0 else int(nh[t])
                        stream_base = int(lo_off[t]) if h == 0 else int(hi_off[t])
                        gb = g_lo if h == 0 else g_hi
                        for i in range(nch):
                            j = stream_base + i
                            k = int(s_off[t]) + ci
                            nc.tensor.matmul(
                                out=pp[:],
                                lhsT=gb[j // BATCH_CH][:, j % BATCH_CH, :],
                                rhs=s_bufs[k // BATCH_CH][:, k % BATCH_CH, :],
                                start=(ci == 0), stop=(ci == ntot - 1),
                            )
                            ci += 1
                    propT = work.tile([P, P], b16, tag="propT")
                    nc.vector.tensor_copy(out=propT[:], in_=pp[:])
                    if debug and l == 0 and t == 0:
                        nc.sync.dma_start(out=dbg_prop[:], in_=propT[:])
                    pz = psum_z.tile([P, nout], f32, tag="pz")
                    nc.tensor.matmul(out=pz[:], lhsT=propT[:], rhs=w_t[l][:],
                                     start=True, stop=False)
                    nc.tensor.matmul(out=pz[:], lhsT=ones_t[:], rhs=b_t[l][:],
                                     start=False, stop=True)
                    if l < 2:
                        ht = work.tile([P, D], f32, tag="ht")
                        nc.scalar.activation(
                            out=ht[:], in_=pz[:],
                            func=mybir.ActivationFunctionType.Relu)
                        sq = work.tile([P, D], f32, tag="sq")
                        ss = work.tile([P, 1], f32, tag="ss")
                        nc.scalar.activation(
                            out=sq[:], in_=ht[:],
                            func=mybir.ActivationFunctionType.Square,
                            accum_out=ss[:])
                        nc.scalar.activation(
                            out=ss[:], in_=ss[:],
                            func=mybir.ActivationFunctionType.Sqrt)
                        nc.vector.tensor_scalar_max(out=ss[:], in0=ss[:],
                                                    scalar1=float(EPS))
                        nc.vector.reciprocal(out=ss[:], in_=ss[:])
                        hb = work.tile([P, D], b16, tag="hb")
                        nc.scalar.activation(
                            out=hb[:], in_=ht[:],
                            func=mybir.ActivationFunctionType.Copy,
                            scale=ss[:])
                        nc.sync.dma_start(
                            out=ag_in[l][t * P:(t + 1) * P, :], in_=hb[:])
                    else:
                        zt = work.tile([P, NCLS], f32, tag="zt")
                        nc.vector.tensor_copy(out=zt[:], in_=pz[:])
                        nc.sync.dma_start(
                            out=out_t[t * P:(t + 1) * P, :], in_=zt[:])
                if l < 2:
                    nc.gpsimd.collective_compute(
                        "AllGather",
                        mybir.AluOpType.bypass,
                        ins=[ag_in[l].opt()],
                        outs=[ag_out[l].opt()],
                        replica_groups=[list(range(NCORES))],
                    )
                    if debug:
                        nc.sync.dma_start(out=dbg_h[l][:], in_=ag_in[l][:])
                        nc.sync.dma_start(out=dbg_ag[l][:], in_=ag_out[l][:])
    nc.compile()
    return nc


_CACHE = {}


def _get_program(sched):
    key = (sched["nchunk"], sched["nbl"], sched["nbh"],
           tuple(sched["nl"]), tuple(sched["nh"]))
    if key not in _CACHE:
        _CACHE[key] = _build(sched)
    return _CACHE[key]


# ---------------------------------------------------------------- entry
def kernel(x, edge_index, C_vals, W1, b1, W2, b2, W3, b3):
    from concourse.bass_utils import run_bass_kernel_spmd

    x = np.asarray(x)
    sched = _prepare(edge_index, C_vals)
    nc = _get_program(sched)

    xbf = np.zeros((PAD_N, D), bf16)
    xbf[:N] = x.astype(bf16)
    common = {
        "xbf": xbf,
        "W1": np.asarray(W1).astype(bf16), "b1": np.asarray(b1).astype(bf16).reshape(1, D),
        "W2": np.asarray(W2).astype(bf16), "b2": np.asarray(b2).astype(bf16).reshape(1, D),
        "W3": np.asarray(W3).astype(bf16), "b3": np.asarray(b3).astype(bf16).reshape(1, NCLS),
    }
    in_maps = []
    for c in range(NCORES):
        m = dict(common)
        m["s_mat"] = sched["s_mat"][c]
        m["idx_lo"] = sched["idx_lo"][c]
        m["idx_hi"] = sched["idx_hi"][c]
        in_maps.append(m)

    trace = bool(int(os.environ.get("GNN_TRACE", "0")))
    kwargs = {}
    if trace:
        import trace_utils
        trace_utils.install()
        kwargs = dict(trace=True, tmpdir="/tmp/gnn_trace")

    res = run_bass_kernel_spmd(nc, in_maps, core_ids=list(range(NCORES)),
                               **kwargs)
    if trace and res.exec_time_ns is not None:
        print(f"HW exec time: {res.exec_time_ns} ns")

    out = np.concatenate([res.results[c]["out"] for c in range(NCORES)], axis=0)
    return np.ascontiguousarray(out[:N])


# revision 8
# speedup vs baseline: 1.3177x; 1.0688x over previous
"""Trainium2 Bass kernel for nn_CGNN (3-layer GNN message passing).

Math per layer:  prop = A @ h  (A sparse COO: out[row] += C * h[col]);
z = prop @ W + b; if not last: h' = l2norm_rows(relu(z)).

Distribution: destination-node sharding across 8 cores (6272 rows each,
49 tiles of 128). Each core gathers source rows h[col] for its edges with
the custom SWDGE dma_gather (4 queues), then performs the segment-sum as
PE matmuls:  propT[f, d] = sum_e G[e, f] * S[e, d]   (S built on host,
C folded in, bf16; edges grouped per dest tile, deduped by source, padded
to 128-chunks, uniform group sizes across cores so one SPMD program
serves all 8 cores).  h is AllGathered between layers (bf16).

Self-contained: hardcodes all shapes from the problem spec.
"""
import os
import sys
import types

import numpy as np
import ml_dtypes

# ---------------------------------------------------------------- constants
N = 50000
E = 800000
D = 128
NCLS = 64
NCORES = 8
P = 128
PAD_N = 50176            # 8 * 6272
SHARD = PAD_N // NCORES  # 6272
NT = SHARD // P          # 49 dest tiles per core
HALF = PAD_N // 2        # 25088 (int16 gather index limit is 32767)
BATCH_CH = 16            # chunks per gather call -> 2048 idx
BATCH = BATCH_CH * P     # 4096
NQ = 4                   # SWDGE queues
EPS = 1e-12

bf16 = ml_dtypes.bfloat16


# ---------------------------------------------------------------- host prep
def _prepare(edge_index, C_vals):
    """Build per-core gather index streams + packed S matrices + a
    core-uniform schedule.

    Returns dict with:
      nl, nh        : [NT] chunks per tile for lo/hi stream (uniform)
      nchunk        : total S chunks per core
      nbl, nbh      : gather batches per stream
      idx_lo, idx_hi: [NCORES][128, Llo/16] int16 wrapped indices
      s_mat         : [NCORES][128, nchunk, 128] bf16
    """
    row = np.asarray(edge_index[0], dtype=np.int64)
    col = np.asarray(edge_index[1], dtype=np.int64)
    C = np.asarray(C_vals, dtype=np.float32)

    core = row // SHARD
    tile_of = (row % SHARD) // P
    dloc = row % P
    half = (col >= HALF).astype(np.int64)

    # group key: (core, tile, half); within group dedup cols
    # first pass: unique source count per (c, t, h)
    ucount = np.zeros((NCORES, NT, 2), np.int64)
    groups = {}
    key = ((core * NT + tile_of) * 2 + half)
    order = np.argsort(key, kind="stable")
    ks = key[order]
    bounds = np.searchsorted(ks, np.arange(NCORES * NT * 2 + 1))
    for c in range(NCORES):
        for t in range(NT):
            for h in (0, 1):
                k = (c * NT + t) * 2 + h
                sel = order[bounds[k]:bounds[k + 1]]
                cols_g = col[sel]
                u, inv = np.unique(cols_g, return_inverse=True)
                groups[(c, t, h)] = (u, inv, dloc[sel], C[sel])
                ucount[c, t, h] = len(u)

    # uniform padded group sizes (multiples of P, max over cores)
    gsz = ((ucount.max(axis=0) + P - 1) // P) * P  # [NT, 2]
    nl = (gsz[:, 0] // P).astype(np.int64)
    nh = (gsz[:, 1] // P).astype(np.int64)
    nchunk = int((nl + nh).sum())
    s_off = np.concatenate([[0], np.cumsum(nl + nh)])[:NT]
    lo_off = np.concatenate([[0], np.cumsum(nl)])[:NT]
    hi_off = np.concatenate([[0], np.cumsum(nh)])[:NT]
    llo_ch = int(nl.sum())
    lhi_ch = int(nh.sum())
    nbl = -(-llo_ch // BATCH_CH)
    nbh = -(-lhi_ch // BATCH_CH)

    idx_lo_all, idx_hi_all, s_all = [], [], []
    for c in range(NCORES):
        s_mat = np.zeros((P, nchunk, P), np.float32)
        str_lo = np.zeros(nbl * BATCH, np.int64)
        str_hi = np.zeros(nbh * BATCH, np.int64)
        for t in range(NT):
            for h in (0, 1):
                u, inv, dl, cv = groups[(c, t, h)]
                base_s = (s_off[t] + (0 if h == 0 else nl[t])) * P
                r = base_s + inv
                np.add.at(s_mat, (r % P, r // P, dl), cv)
                stream = str_lo if h == 0 else str_hi
                boff = (lo_off[t] if h == 0 else hi_off[t]) * P
                stream[boff:boff + len(u)] = u - (0 if h == 0 else HALF)
        s_all.append(s_mat.astype(bf16))
        idx_lo_all.append(_wrap_idxs(str_lo))
        idx_hi_all.append(_wrap_idxs(str_hi))

    return {
        "nl": nl, "nh": nh, "nchunk": nchunk, "nbl": nbl, "nbh": nbh,
        "s_off": s_off, "lo_off": lo_off, "hi_off": hi_off,
        "idx_lo": idx_lo_all, "idx_hi": idx_hi_all, "s_mat": s_all,
    }


def _wrap_idxs(idx):
    """[L] -> [128, L/16] int16 wrapped (pos i = s*16 + p), replicated x8."""
    n = idx.shape[0]
    assert n % 16 == 0
    w = idx.astype(np.int16).reshape(n // 16, 16).T
    return np.ascontiguousarray(np.tile(w, (8, 1)))


# ---------------------------------------------------------------- device
def _build(sched):
    import concourse.bass as bass
    import concourse.bacc as bacc
    import concourse.mybir as mybir
    import concourse.tile as tile
    from concourse import library_config

    nl, nh = sched["nl"], sched["nh"]
    nchunk, nbl, nbh = sched["nchunk"], sched["nbl"], sched["nbh"]
    s_off, lo_off, hi_off = sched["s_off"], sched["lo_off"], sched["hi_off"]
    nsb = -(-nchunk // BATCH_CH)

    f32 = mybir.dt.float32
    b16 = mybir.dt.bfloat16

    nc = bacc.Bacc("TRN2", num_devices=NCORES, num_swdge_queues=NQ)
    xbf = nc.dram_tensor("xbf", [PAD_N, D], b16, kind="ExternalInput")
    s_in = nc.dram_tensor("s_mat", [P, nchunk, P], b16, kind="ExternalInput")
    ilo = nc.dram_tensor("idx_lo", [P, nbl * BATCH // 16], mybir.dt.int16,
                         kind="ExternalInput")
    ihi = nc.dram_tensor("idx_hi", [P, nbh * BATCH // 16], mybir.dt.int16,
                         kind="ExternalInput")
    w_in = [nc.dram_tensor(f"W{i+1}", [D, D if i < 2 else NCLS], b16,
                           kind="ExternalInput") for i in range(3)]
    b_in = [nc.dram_tensor(f"b{i+1}", [1, D if i < 2 else NCLS], b16,
                           kind="ExternalInput") for i in range(3)]
    out_t = nc.dram_tensor("out", [SHARD, NCLS], f32, kind="ExternalOutput")
    debug = bool(int(os.environ.get("GNN_DEBUG", "0")))
    if debug:
        dbg_h = [nc.dram_tensor(f"dbg_h{l+1}", [SHARD, D], b16,
                                kind="ExternalOutput") for l in range(2)]
        dbg_ag = [nc.dram_tensor(f"dbg_ag{l+1}", [PAD_N, D], b16,
                                 kind="ExternalOutput") for l in range(2)]
        dbg_prop = nc.dram_tensor("dbg_prop", [P, P], b16, kind="ExternalOutput")
        dbg_g = nc.dram_tensor("dbg_g", [P, BATCH_CH, D], b16, kind="ExternalOutput")

    with tile.TileContext(nc) as tc:
        nc.gpsimd.load_library(library_config.mlp)
        with (
            tc.tile_pool(name="dram", bufs=1, space="DRAM") as dram,
            tc.tile_pool(name="singles", bufs=1) as singles,
            tc.tile_pool(name="glo", bufs=10) as glo_pool,
            tc.tile_pool(name="ghi", bufs=10) as ghi_pool,
            tc.tile_pool(name="sbat", bufs=4) as sb_pool,
            tc.tile_pool(name="work", bufs=6) as work,
            tc.tile_pool(name="psum_p", bufs=3, space="PSUM") as psum_p,
            tc.tile_pool(name="psum_z", bufs=3, space="PSUM") as psum_z,
        ):
            ag_in = [dram.tile([SHARD, D], b16, name=f"ag_in{l}", tag=f"ag_in{l}") for l in range(2)]
            ag_out = [dram.tile([PAD_N, D], b16, name=f"ag_out{l}", tag=f"ag_out{l}") for l in range(2)]

            idx_lo_t = singles.tile([P, nbl * BATCH // 16], mybir.dt.int16,
                                    tag="idxlo")
            idx_hi_t = singles.tile([P, nbh * BATCH // 16], mybir.dt.int16,
                                    tag="idxhi")
            nc.sync.dma_start(out=idx_lo_t[:], in_=ilo[:])
            nc.sync.dma_start(out=idx_hi_t[:], in_=ihi[:])
            w_t, b_t = [], []
            for i in range(3):
                nout = D if i < 2 else NCLS
                wt = singles.tile([D, nout], b16, name=f"w{i}", tag=f"w{i}")
                bt = singles.tile([1, nout], b16, name=f"b{i}", tag=f"b{i}")
                nc.sync.dma_start(out=wt[:], in_=w_in[i][:])
                nc.sync.dma_start(out=bt[:], in_=b_in[i][:])
                w_t.append(wt)
                b_t.append(bt)
            ones_t = singles.tile([1, P], b16, tag="ones")
            nc.vector.memset(ones_t[:], 1.0)

            qrr = [0]

            def issue_gather(table_ap, idx_tile, b, pool):
                g = pool.tile([P, BATCH_CH, D], b16)
                nc.gpsimd.dma_gather(
                    g[:], table_ap,
                    idx_tile[:, (b * BATCH // 16):((b + 1) * BATCH // 16)],
                    BATCH, BATCH, D,
                    single_packet=False, queue_num=qrr[0] % NQ,
                )
                qrr[0] += 1
                return g

            for l in range(3):
                nout = D if l < 2 else NCLS
                if l == 0:
                    tbl = xbf
                else:
                    tbl = ag_out[l - 1]
                tbl_lo = tbl[0:HALF, :]
                tbl_hi = tbl[HALF:PAD_N, :]

                # interleave lo/hi gather issue so tiles complete progressively
                g_lo, g_hi = [], []
                for b in range(max(nbl, nbh)):
                    if b < nbl:
                        g_lo.append(issue_gather(tbl_lo, idx_lo_t, b, glo_pool))
                    if b < nbh:
                        g_hi.append(issue_gather(tbl_hi, idx_hi_t, b, ghi_pool))
                if debug and l == 1:
                    nc.sync.dma_start(out=dbg_g[:], in_=g_lo[0][:])
                s_bufs = []
                for b in range(nsb):
                    c0 = b * BATCH_CH
                    c1 = min(nchunk, c0 + BATCH_CH)
                    sb = sb_pool.tile([P, BATCH_CH, P], b16)
                    nc.sync.dma_start(out=sb[:, 0:(c1 - c0), :],
                                      in_=s_in[:, c0:c1, :])
                    s_bufs.append(sb)

                for t in range(NT):
                    pp = psum_p.tile([P, P], f32)
                    ntot = int(nl[t] + nh[t])
                    ci = 0
                    for h in (0, 1):
                        nch = int(nl[t]) if h == 0 else int(nh[t])
                        stream_base = int(lo_off[t]) if h == 0 else int(hi_off[t])
                        gb = g_lo if h == 0 else g_hi
                        for i in range(nch):
                            j = stream_base + i
                            k = int(s_off[t]) + ci
                            nc.tensor.matmul(
                                out=pp[:],
                                lhsT=gb[j // BATCH_CH][:, j % BATCH_CH, :],
                                rhs=s_bufs[k // BATCH_CH][:, k % BATCH_CH, :],
                                start=(ci == 0), stop=(ci == ntot - 1),
                            )
                            ci += 1
                    propT = work.tile([P, P], b16, tag="propT")
                    nc.vector.tensor_copy(out=propT[:], in_=pp[:])
                    if debug and l == 0 and t == 0:
                        nc.sync.dma_start(out=dbg_prop[:], in_=propT[:])
                    pz = psum_z.tile([P, nout], f32, tag="pz")
                    nc.tensor.matmul(out=pz[:], lhsT=propT[:], rhs=w_t[l][:],
                                     start=True, stop=False)
                    nc.tensor.matmul(out=pz[:], lhsT=ones_t[:], rhs=b_t[l][:],
                                     start=False, stop=True)
                    if l < 2:
                        ht = work.tile([P, D], f32, tag="ht")
                        nc.scalar.activation(
                            out=ht[:], in_=pz[:],
                            func=mybir.ActivationFunctionType.Relu)
                        sq = work.tile([P, D], f32, tag="sq")
                        ss = work.tile([P, 1], f32, tag="ss")
                        nc.scalar.activation(
                            out=sq[:], in_=ht[:],
                            func=mybir.ActivationFunctionType.Square,
                            accum_out=ss[:])
                        nc.scalar.activation(
                            out=ss[:], in_=ss[:],
                            func=mybir.ActivationFunctionType.Sqrt)
                        nc.vector.tensor_scalar_max(out=ss[:], in0=ss[:],
                                                    scalar1=float(EPS))
                        nc.vector.reciprocal(out=ss[:], in_=ss[:])
                        hb = work.tile([P, D], b16, tag="hb")
                        nc.scalar.activation(
                            out=hb[:], in_=ht[:],
                            func=mybir.ActivationFunctionType.Copy,
                            scale=ss[:])
                        nc.sync.dma_start(
                            out=ag_in[l][t * P:(t + 1) * P, :], in_=hb[:])
                    else:
                        zt = work.tile([P, NCLS], f32, tag="zt")
                        nc.vector.tensor_copy(out=zt[:], in_=pz[:])
                        nc.sync.dma_start(
                            out=out_t[t * P:(t + 1) * P, :], in_=zt[:])
                if l < 2:
                    nc.gpsimd.collective_compute(
                        "AllGather",
                        mybir.AluOpType.bypass,
                        ins=[ag_in[l].opt()],
                        outs=[ag_out[l].opt()],
                        replica_groups=[list(range(NCORES))],
                    )
                    if debug:
                        nc.sync.dma_start(out=dbg_h[l][:], in_=ag_in[l][:])
                        nc.sync.dma_start(out=dbg_ag[l][:], in_=ag_out[l][:])
    nc.compile()
    return nc


_CACHE = {}


def _get_program(sched):
    key = (sched["nchunk"], sched["nbl"], sched["nbh"],
           tuple(sched["nl"]), tuple(sched["nh"]))
    if key not in _CACHE:
        _CACHE[key] = _build(sched)
    return _CACHE[key]


# ---------------------------------------------------------------- entry
def kernel(x, edge_index, C_vals, W1, b1, W2, b2, W3, b3):
    from concourse.bass_utils import run_bass_kernel_spmd

    x = np.asarray(x)
    sched = _prepare(edge_index, C_vals)
    nc = _get_program(sched)

    xbf = np.zeros((PAD_N, D), bf16)
    xbf[:N] = x.astype(bf16)
    common = {
        "xbf": xbf,
        "W1": np.asarray(W1).astype(bf16), "b1": np.asarray(b1).astype(bf16).reshape(1, D),
        "W2": np.asarray(W2).astype(bf16), "b2": np.asarray(b2).astype(bf16).reshape(1, D),
        "W3": np.asarray(W3).astype(bf16), "b3": np.asarray(b3).astype(bf16).reshape(1, NCLS),
    }
    in_maps = []
    for c in range(NCORES):
        m = dict(common)
        m["s_mat"] = sched["s_mat"][c]
        m["idx_lo"] = sched["idx_lo"][c]
        m["idx_hi"] = sched["idx_hi"][c]
        in_maps.append(m)

    trace = bool(int(os.environ.get("GNN_TRACE", "0")))
    kwargs = {}
    if trace:
        import trace_utils
        trace_utils.install()
        kwargs = dict(trace=True, tmpdir="/tmp/gnn_trace")

    res = run_bass_kernel_spmd(nc, in_maps, core_ids=list(range(NCORES)),
                               **kwargs)
    if trace and res.exec_time_ns is not None:
        print(f"HW exec time: {res.exec_time_ns} ns")

    out = np.concatenate([res.results[c]["out"] for c in range(NCORES)], axis=0)
    return np.ascontiguousarray(out[:N])
